# revision 14
# baseline (speedup 1.0000x reference)
"""Trainium2 Bass kernel for nn_DegreePrediction (RBC via batched Perron vectors).

Math: M[s,t] = weights_r*r_zeros + r_const is positive column-stochastic
(columns sum to 1); its eigenvalue-1 right eigenvector is the Perron
vector and rbc[n] = sum_{s,t} T[s,t]/v[s,t,s] * v[s,t,n] is scale-free in
v.  v ~= M^2 @ ones to ~lambda2^2 ~ 0.4% << the 2e-2 gate, so two batched
mat-vec sweeps suffice (no squarings, no transposes).

Layout trick: each core's 512 matrices are uploaded TRANSPOSED in bf16,
two per 128-partition stack: MT[j+64h, 64q+i] = M_{2q+h}[i,j].  With
lhsT = a [128,128] MT block (stationary operand) both sweeps keep their
results in the PARTITION dim:
  pass A: rhs = ones-blocks [128,2]       -> out[m,n] = rowsums w_p[m]
  pass B: rhs = block-diag w cols [128,4] -> out[m,n] = v_p[m]
LDWEIGHTS/MATMUL pairs pipeline through the PE reorder window (~30ns per
block), so the kernel is DMA-paced: chunks are stored contiguously in
DRAM and streamed in order, and the pipeline is split in column halves
so pass B of half 0 and its tail overlap the DMA of half 1.  The
denominator row v_p[s_p] is gathered with a host mask + ones-matmul;
reciprocal runs on ACT (table preloaded during the DMA window; the DVE
iterative divide on a 1-partition row costs 3.3us).

Sharding: pairs split by s across 8 cores; host sums the partials.
"""

import numpy as np

_N = 64
_NCORES = 8
_NP = 512          # pairs per core
_NQ = 128          # double-stacks (4 pairs each)
_NCHUNK = 8        # DMA chunks of MT
_CW = _NQ * 128 // _NCHUNK   # MT cols per chunk (2048)
_QPC = _NQ // _NCHUNK        # double-stacks per chunk (16)

_cached = {}


def _build_program():
    import concourse.tile as tile
    from concourse import bacc, mybir
    from contextlib import ExitStack

    f32 = mybir.dt.float32
    bf16 = mybir.dt.bfloat16
    fp8 = mybir.dt.float8e4
    AF = mybir.ActivationFunctionType
    nc = bacc.Bacc("TRN2", target_bir_lowering=False, debug=False)
    mt_in = nc.dram_tensor("mt", [_NCHUNK, 128, _CW], bf16,
                           kind="ExternalInput").ap()
    maskd_in = nc.dram_tensor("maskd", [128, _NP], f32, kind="ExternalInput").ap()
    tmt_in = nc.dram_tensor("tmt", [1, _NP], f32, kind="ExternalInput").ap()
    tmb_in = nc.dram_tensor("tmb", [1, _NP], f32, kind="ExternalInput").ap()
    e2_in = nc.dram_tensor("e2", [128, _N], f32, kind="ExternalInput").ap()
    out_dram = nc.dram_tensor("out", [1, _N], f32, kind="ExternalOutput").ap()

    with tile.TileContext(nc) as tc:
        with ExitStack() as ctx:
            consts = ctx.enter_context(tc.tile_pool(name="consts", bufs=1))
            psum = ctx.enter_context(tc.tile_pool(name="psum", bufs=1, space="PSUM"))

            # ---- stream MT chunks first (contiguous, in order) ----
            mtc = []
            for d in range(_NCHUNK):
                t = consts.tile([128, _CW], bf16, tag=f"mt{d}")
                nc.sync.dma_start(out=t[:, :], in_=mt_in[d, :, :])
                mtc.append(t)

            # ---- small inputs (tail-only) ----
            maskd = consts.tile([128, _NP], f32)
            nc.sync.dma_start(out=maskd[:, :], in_=maskd_in[:, :])
            tmt = consts.tile([1, _NP], f32)
            nc.sync.dma_start(out=tmt[:, :], in_=tmt_in[:, :])
            tmb = consts.tile([1, _NP], f32)
            nc.sync.dma_start(out=tmb[:, :], in_=tmb_in[:, :])
            e2 = consts.tile([128, _N], f32)
            nc.sync.dma_start(out=e2[:, :], in_=e2_in[:, :])

            ones2 = consts.tile([128, 2], bf16)
            nc.vector.memset(ones2[:, :], 0.0)
            nc.vector.memset(ones2[0:64, 0:1], 1.0)
            nc.vector.memset(ones2[64:128, 1:2], 1.0)
            ones128 = consts.tile([128, 1], bf16)
            nc.vector.memset(ones128[:, :], 1.0)
            one1 = consts.tile([1, 1], f32)
            nc.vector.memset(one1[:, :], 1.0)
            etop = consts.tile([1, 128], bf16)
            nc.vector.memset(etop[:, :], 0.0)
            nc.vector.memset(etop[0:1, 0:64], 1.0)
            ebot = consts.tile([1, 128], bf16)
            nc.vector.memset(ebot[:, :], 0.0)
            nc.vector.memset(ebot[0:1, 64:128], 1.0)
            L = consts.tile([128, 4 * _NQ], bf16)
            nc.vector.memset(L[:, :], 0.0)
            # hoist the ACT ln/exp table load into the DMA window
            scratch = consts.tile([1, 1], f32)
            with tc.high_priority():
                nc.scalar.activation(out=scratch[:, :], in_=one1[:, :],
                                     func=AF.Ln)

            JNK = psum.tile([1, 512], f32, tag="JNK")

            def keep_warm(d):
                nc.tensor.matmul(out=JNK[:, :], lhsT=ones128[:, :],
                                 rhs=mtc[d][:, 0:512], start=True, stop=True)

            WW = psum.tile([128, 2 * _NQ], f32, tag="WW")
            VV = psum.tile([128, _NP], f32, tag="VV")
            DPS = psum.tile([1, _NP], f32, tag="DPS")
            CB = psum.tile([128, _NP], f32, tag="CB")
            WWv = WW[:, :].rearrange("p (q two) -> p q two", two=2)
            Lv = L[:, :].rearrange("p (q four) -> p q four", four=4)
            dmm = consts.tile([128, _NP], bf16)
            dinv = consts.tile([1, _NP], f32)
            ct = consts.tile([1, _NP], bf16)
            cb = consts.tile([1, _NP], bf16)
            cbs = consts.tile([128, _NP], f32)
            vc = consts.tile([128, _NP], f32)
            r1h = []
            for h in (0, 1):
                r1t = consts.tile([128, 1], f32, tag=f"r1{h}")
                r1h.append(r1t)

            def sweepA(h):
                for Q in range(64 * h, 64 * h + 64):
                    d, r = Q // _QPC, Q % _QPC
                    if Q % 4 == 0:
                        keep_warm(d)
                    nc.tensor.matmul(
                        out=WW[:, 2 * Q:2 * Q + 2],
                        lhsT=mtc[d][:, 128 * r:128 * r + 128],
                        rhs=ones2[:, :], start=True, stop=True)

            def lbuild(h):
                qs = slice(64 * h, 64 * h + 64)
                nc.vector.tensor_copy(out=Lv[0:64, qs, 0], in_=WWv[0:64, qs, 0])
                nc.vector.tensor_copy(out=Lv[64:128, qs, 1], in_=WWv[0:64, qs, 1])
                nc.vector.tensor_copy(out=Lv[0:64, qs, 2], in_=WWv[64:128, qs, 0])
                nc.vector.tensor_copy(out=Lv[64:128, qs, 3], in_=WWv[64:128, qs, 1])

            def sweepB(h):
                for Q in range(64 * h, 64 * h + 64):
                    d, r = Q // _QPC, Q % _QPC
                    if Q % 4 == 0:
                        keep_warm(d)
                    nc.tensor.matmul(
                        out=VV[:, 4 * Q:4 * Q + 4],
                        lhsT=mtc[d][:, 128 * r:128 * r + 128],
                        rhs=L[:, 4 * Q:4 * Q + 4], start=True, stop=True)

            def tail_dve(h):
                sl = slice(256 * h, 256 * h + 256)
                nc.vector.tensor_mul(out=dmm[:, sl], in0=VV[:, sl],
                                     in1=maskd[:, sl])

            def tail_pe_d(h):
                sl = slice(256 * h, 256 * h + 256)
                nc.tensor.matmul(out=DPS[:, sl], lhsT=ones128[:, :],
                                 rhs=dmm[:, sl], start=True, stop=True)

            def tail_coef(h):
                sl = slice(256 * h, 256 * h + 256)
                # 1/d = exp(-ln d) on ACT: d > 0 (Perron), and the DVE
                # iterative divide on a 1-partition row costs 8 cyc/elem.
                nc.scalar.activation(out=dinv[:, sl], in_=DPS[:, sl],
                                     func=AF.Ln)
                nc.scalar.activation(out=dinv[:, sl], in_=dinv[:, sl],
                                     func=AF.Exp, scale=-1.0)
                nc.vector.tensor_mul(out=ct[:, sl], in0=tmt[:, sl],
                                     in1=dinv[:, sl])
                nc.vector.tensor_mul(out=cb[:, sl], in0=tmb[:, sl],
                                     in1=dinv[:, sl])

            def tail_pe_cb(h):
                sl = slice(256 * h, 256 * h + 256)
                nc.tensor.matmul(out=CB[:, sl], lhsT=etop[:, :], rhs=ct[:, sl],
                                 start=True, stop=False)
                nc.tensor.matmul(out=CB[:, sl], lhsT=ebot[:, :], rhs=cb[:, sl],
                                 start=False, stop=True)

            def tail_fin(h):
                sl = slice(256 * h, 256 * h + 256)
                nc.scalar.copy(out=cbs[:, sl], in_=CB[:, sl])
                nc.vector.tensor_mul(out=vc[:, sl], in0=VV[:, sl],
                                     in1=cbs[:, sl])
                nc.vector.tensor_reduce(
                    out=r1h[h][:, :], in_=vc[:, sl],
                    axis=mybir.AxisListType.X, op=mybir.AluOpType.add)

            sweepA(0)
            lbuild(0)
            sweepB(0)
            tail_dve(0)
            sweepA(1)          # PE: runs while half-0 tail DVE work proceeds
            tail_pe_d(0)
            lbuild(1)
            tail_coef(0)
            tail_pe_cb(0)
            sweepB(1)
            tail_fin(0)
            tail_dve(1)
            tail_pe_d(1)
            tail_coef(1)
            tail_pe_cb(1)
            tail_fin(1)

            r1 = consts.tile([128, 1], f32)
            nc.vector.tensor_add(out=r1[:, :], in0=r1h[0][:, :], in1=r1h[1][:, :])
            # fold halves AND transpose to a row in one matmul:
            # FR[0,n] = sum_k r1[k]*E2[k,n] = r1[n] + r1[64+n]
            FR = psum.tile([1, _N], f32, tag="FR")
            nc.tensor.matmul(out=FR[:, :], lhsT=r1[:, :], rhs=e2[:, :],
                             start=True, stop=True)
            out_sb = consts.tile([1, _N], f32)
            nc.scalar.copy(out=out_sb[:, :], in_=FR[:, :])
            nc.sync.dma_start(out=out_dram[:, :], in_=out_sb[:, :])
    nc.compile()
    return nc


def _get_program():
    if "nc" not in _cached:
        _cached["nc"] = _build_program()
    return _cached["nc"]


def _build_in_maps(x, weights_t, r_const):
    """Host-side layouts for all 8 cores."""
    import ml_dtypes

    M_all = r_const.reshape(_N * _N, _N, _N)
    i = np.arange(_N)
    r_diag = r_const[i[:, None], i[None, :], i[:, None], i[:, None]]
    T_full = (x * weights_t * r_diag).astype(np.float32)      # [64, 64]

    e2 = np.zeros((128, _N), np.float32)
    e2[np.arange(128), np.arange(128) % _N] = 1.0

    p = np.arange(_NP)
    b = (p >> 1) & 1                                          # stack-half of pair
    s_loc = p >> 6
    t_loc = p & 63

    in_maps = []
    for c in range(_NCORES):
        Mc = np.asarray(M_all[_NP * c:_NP * (c + 1)], np.float32)  # (p,i,j)
        # MT[j+64h, 64(2Q+b)+i] = Mc[4Q+2b+h, i, j], then chunked contiguously
        mt = (Mc.reshape(_NQ, 2, 2, _N, _N)       # (Q, b, h, i, j)
              .transpose(2, 4, 0, 1, 3)           # (h, j, Q, b, i)
              .reshape(128, _NQ * 128))
        mt = (mt.reshape(128, _NCHUNK, _CW).transpose(1, 0, 2))  # (chunk, p, f)
        mt = np.ascontiguousarray(mt).astype(ml_dtypes.bfloat16)

        maskd = np.zeros((128, _NP), np.float32)
        maskd[64 * b + 8 * c + s_loc, p] = 1.0   # v's node index is GLOBAL s

        Tp = T_full[8 * c + s_loc, t_loc]                     # [512]
        tmt = np.where(b == 0, Tp, 0.0).astype(np.float32)[None, :]
        tmb = np.where(b == 1, Tp, 0.0).astype(np.float32)[None, :]

        in_maps.append({"mt": mt, "maskd": maskd,
                        "tmt": np.ascontiguousarray(tmt),
                        "tmb": np.ascontiguousarray(tmb), "e2": e2})
    return in_maps


def kernel(x, weights_t, weights_r, r_zeros, r_const):
    from concourse.bass_utils import run_bass_kernel_spmd

    x = np.asarray(x, np.float32)
    weights_t = np.asarray(weights_t, np.float32)
    r_const = np.asarray(r_const, np.float32)
    r_zeros_np = np.asarray(r_zeros)
    if np.any(r_zeros_np):
        r_const = (np.asarray(weights_r, np.float32)
                   * r_zeros_np.astype(np.float32) + r_const)

    nc = _get_program()
    in_maps = _build_in_maps(x, weights_t, r_const)
    res = run_bass_kernel_spmd(nc, in_maps, core_ids=list(range(_NCORES)))
    parts = np.stack([r["out"][0, :] for r in res.results])  # [8, 64]
    return parts.sum(axis=0, dtype=np.float64).astype(np.float32)


# revision 15
# speedup vs baseline: 1.4340x; 1.4340x over previous
"""Trainium2 Bass kernel for nn_DegreePrediction (RBC via batched Perron vectors).

Math: M[s,t] = weights_r*r_zeros + r_const is positive column-stochastic
(columns sum to 1); its eigenvalue-1 right eigenvector is the Perron
vector and rbc[n] = sum_{s,t} T[s,t]/v[s,t,s] * v[s,t,n] is scale-free in
v.  v ~= M^2 @ ones to ~lambda2^2 ~ 0.4% << the 2e-2 gate, so two batched
mat-vec sweeps suffice (no squarings, no transposes).

Layout trick: each core's 512 matrices are uploaded TRANSPOSED in bf16,
two per 128-partition stack: MT[j+64h, 64q+i] = M_{2q+h}[i,j].  With
lhsT = a [128,128] MT block (stationary operand) both sweeps keep their
results in the PARTITION dim:
  pass A: rhs = ones-blocks [128,2]       -> out[m,n] = rowsums w_p[m]
  pass B: rhs = block-diag w cols [128,4] -> out[m,n] = v_p[m]
LDWEIGHTS/MATMUL pairs pipeline through the PE reorder window (~30ns per
block), so the kernel is DMA-paced: chunks are stored contiguously in
DRAM and streamed in order, and the pipeline is split in column halves
so pass B of half 0 and its tail overlap the DMA of half 1.  The
denominator row v_p[s_p] is gathered with a host mask + ones-matmul;
reciprocal runs on ACT (table preloaded during the DMA window; the DVE
iterative divide on a 1-partition row costs 3.3us).

Sharding: pairs split by s across 8 cores; host sums the partials.
"""

import numpy as np

_N = 64
_NCORES = 8
_NP = 512          # pairs per core
_NQ = 128          # double-stacks (4 pairs each)
_NCHUNK = 8        # DMA chunks of MT
_CW = _NQ * 128 // _NCHUNK   # MT cols per chunk (2048)
_QPC = _NQ // _NCHUNK        # double-stacks per chunk (16)

_cached = {}


def _build_program():
    import concourse.tile as tile
    from concourse import bacc, mybir
    from contextlib import ExitStack

    f32 = mybir.dt.float32
    bf16 = mybir.dt.bfloat16
    fp8 = mybir.dt.float8e4
    AF = mybir.ActivationFunctionType
    nc = bacc.Bacc("TRN2", target_bir_lowering=False, debug=False)
    mt_in = nc.dram_tensor("mt", [_NCHUNK, 128, _CW], bf16,
                           kind="ExternalInput").ap()
    maskd_in = nc.dram_tensor("maskd", [128, _NP], f32, kind="ExternalInput").ap()
    tmt_in = nc.dram_tensor("tmt", [1, _NP], f32, kind="ExternalInput").ap()
    tmb_in = nc.dram_tensor("tmb", [1, _NP], f32, kind="ExternalInput").ap()
    e2_in = nc.dram_tensor("e2", [128, _N], f32, kind="ExternalInput").ap()
    out_dram = nc.dram_tensor("out", [1, _N], f32, kind="ExternalOutput").ap()

    with tile.TileContext(nc) as tc:
        with ExitStack() as ctx:
            consts = ctx.enter_context(tc.tile_pool(name="consts", bufs=1))
            psum = ctx.enter_context(tc.tile_pool(name="psum", bufs=1, space="PSUM"))

            # ---- stream MT chunks first (contiguous, in order) ----
            mtc = []
            for d in range(_NCHUNK):
                t = consts.tile([128, _CW], bf16, tag=f"mt{d}")
                nc.sync.dma_start(out=t[:, :], in_=mt_in[d, :, :])
                mtc.append(t)

            # ---- small inputs (tail-only) ----
            maskd = consts.tile([128, _NP], f32)
            nc.sync.dma_start(out=maskd[:, :], in_=maskd_in[:, :])
            tmt = consts.tile([1, _NP], f32)
            nc.sync.dma_start(out=tmt[:, :], in_=tmt_in[:, :])
            tmb = consts.tile([1, _NP], f32)
            nc.sync.dma_start(out=tmb[:, :], in_=tmb_in[:, :])
            e2 = consts.tile([128, _N], f32)
            nc.sync.dma_start(out=e2[:, :], in_=e2_in[:, :])

            ones2 = consts.tile([128, 2], bf16)
            nc.vector.memset(ones2[:, :], 0.0)
            nc.vector.memset(ones2[0:64, 0:1], 1.0)
            nc.vector.memset(ones2[64:128, 1:2], 1.0)
            ones128 = consts.tile([128, 1], bf16)
            nc.vector.memset(ones128[:, :], 1.0)
            one1 = consts.tile([1, 1], f32)
            nc.vector.memset(one1[:, :], 1.0)
            etop = consts.tile([1, 128], bf16)
            nc.vector.memset(etop[:, :], 0.0)
            nc.vector.memset(etop[0:1, 0:64], 1.0)
            ebot = consts.tile([1, 128], bf16)
            nc.vector.memset(ebot[:, :], 0.0)
            nc.vector.memset(ebot[0:1, 64:128], 1.0)
            L = consts.tile([128, 4 * _NQ], bf16)
            nc.vector.memset(L[:, :], 0.0)
            # hoist the ACT ln/exp table load into the DMA window
            scratch = consts.tile([1, 1], f32)
            with tc.high_priority():
                nc.scalar.activation(out=scratch[:, :], in_=one1[:, :],
                                     func=AF.Ln)

            WW = psum.tile([128, 2 * _NQ], f32, tag="WW")
            VV = psum.tile([128, _NP], f32, tag="VV")
            DPS = psum.tile([1, _NP], f32, tag="DPS")
            CB = psum.tile([128, _NP], f32, tag="CB")
            WWv = WW[:, :].rearrange("p (q two) -> p q two", two=2)
            Lv = L[:, :].rearrange("p (q four) -> p q four", four=4)
            dmm = consts.tile([128, _NP], bf16)
            dinv = consts.tile([1, _NP], f32)
            ct = consts.tile([1, _NP], bf16)
            cb = consts.tile([1, _NP], bf16)
            cbs = consts.tile([128, _NP], f32)
            vc = consts.tile([128, _NP], f32)
            r1h = []
            for h in (0, 1):
                r1t = consts.tile([128, 1], f32, tag=f"r1{h}")
                r1h.append(r1t)

            def sweepA(h):
                for Q in range(64 * h, 64 * h + 64):
                    d, r = Q // _QPC, Q % _QPC
                    nc.tensor.matmul(
                        out=WW[:, 2 * Q:2 * Q + 2],
                        lhsT=mtc[d][:, 128 * r:128 * r + 128],
                        rhs=ones2[:, :], start=True, stop=True)

            def lbuild(h):
                qs = slice(64 * h, 64 * h + 64)
                nc.vector.tensor_copy(out=Lv[0:64, qs, 0], in_=WWv[0:64, qs, 0])
                nc.vector.tensor_copy(out=Lv[64:128, qs, 1], in_=WWv[0:64, qs, 1])
                nc.vector.tensor_copy(out=Lv[0:64, qs, 2], in_=WWv[64:128, qs, 0])
                nc.vector.tensor_copy(out=Lv[64:128, qs, 3], in_=WWv[64:128, qs, 1])

            def sweepB(h):
                for Q in range(64 * h, 64 * h + 64):
                    d, r = Q // _QPC, Q % _QPC
                    nc.tensor.matmul(
                        out=VV[:, 4 * Q:4 * Q + 4],
                        lhsT=mtc[d][:, 128 * r:128 * r + 128],
                        rhs=L[:, 4 * Q:4 * Q + 4], start=True, stop=True)

            def tail_dve(h):
                sl = slice(256 * h, 256 * h + 256)
                nc.vector.tensor_mul(out=dmm[:, sl], in0=VV[:, sl],
                                     in1=maskd[:, sl])

            def tail_pe_d(h):
                sl = slice(256 * h, 256 * h + 256)
                nc.tensor.matmul(out=DPS[:, sl], lhsT=ones128[:, :],
                                 rhs=dmm[:, sl], start=True, stop=True)

            def tail_coef(h):
                sl = slice(256 * h, 256 * h + 256)
                # 1/d = exp(-ln d) on ACT: d > 0 (Perron), and the DVE
                # iterative divide on a 1-partition row costs 8 cyc/elem.
                nc.scalar.activation(out=dinv[:, sl], in_=DPS[:, sl],
                                     func=AF.Ln)
                nc.scalar.activation(out=dinv[:, sl], in_=dinv[:, sl],
                                     func=AF.Exp, scale=-1.0)
                nc.vector.tensor_mul(out=ct[:, sl], in0=tmt[:, sl],
                                     in1=dinv[:, sl])
                nc.vector.tensor_mul(out=cb[:, sl], in0=tmb[:, sl],
                                     in1=dinv[:, sl])

            def tail_pe_cb(h):
                sl = slice(256 * h, 256 * h + 256)
                nc.tensor.matmul(out=CB[:, sl], lhsT=etop[:, :], rhs=ct[:, sl],
                                 start=True, stop=False)
                nc.tensor.matmul(out=CB[:, sl], lhsT=ebot[:, :], rhs=cb[:, sl],
                                 start=False, stop=True)

            def tail_fin(h):
                sl = slice(256 * h, 256 * h + 256)
                nc.scalar.copy(out=cbs[:, sl], in_=CB[:, sl])
                nc.vector.tensor_mul(out=vc[:, sl], in0=VV[:, sl],
                                     in1=cbs[:, sl])
                nc.vector.tensor_reduce(
                    out=r1h[h][:, :], in_=vc[:, sl],
                    axis=mybir.AxisListType.X, op=mybir.AluOpType.add)

            sweepA(0)
            lbuild(0)
            sweepB(0)
            tail_dve(0)
            sweepA(1)          # PE: runs while half-0 tail DVE work proceeds
            tail_pe_d(0)
            lbuild(1)
            tail_coef(0)
            tail_pe_cb(0)
            sweepB(1)
            tail_fin(0)
            tail_dve(1)
            tail_pe_d(1)
            tail_coef(1)
            tail_pe_cb(1)
            tail_fin(1)

            r1 = consts.tile([128, 1], f32)
            nc.vector.tensor_add(out=r1[:, :], in0=r1h[0][:, :], in1=r1h[1][:, :])
            # fold halves AND transpose to a row in one matmul:
            # FR[0,n] = sum_k r1[k]*E2[k,n] = r1[n] + r1[64+n]
            FR = psum.tile([1, _N], f32, tag="FR")
            nc.tensor.matmul(out=FR[:, :], lhsT=r1[:, :], rhs=e2[:, :],
                             start=True, stop=True)
            out_sb = consts.tile([1, _N], f32)
            nc.scalar.copy(out=out_sb[:, :], in_=FR[:, :])
            nc.sync.dma_start(out=out_dram[:, :], in_=out_sb[:, :])
    nc.compile()
    return nc


def _get_program():
    if "nc" not in _cached:
        _cached["nc"] = _build_program()
    return _cached["nc"]


def _build_in_maps(x, weights_t, r_const):
    """Host-side layouts for all 8 cores."""
    import ml_dtypes

    M_all = r_const.reshape(_N * _N, _N, _N)
    i = np.arange(_N)
    r_diag = r_const[i[:, None], i[None, :], i[:, None], i[:, None]]
    T_full = (x * weights_t * r_diag).astype(np.float32)      # [64, 64]

    e2 = np.zeros((128, _N), np.float32)
    e2[np.arange(128), np.arange(128) % _N] = 1.0

    p = np.arange(_NP)
    b = (p >> 1) & 1                                          # stack-half of pair
    s_loc = p >> 6
    t_loc = p & 63

    in_maps = []
    for c in range(_NCORES):
        Mc = np.asarray(M_all[_NP * c:_NP * (c + 1)], np.float32)  # (p,i,j)
        # MT[j+64h, 64(2Q+b)+i] = Mc[4Q+2b+h, i, j], then chunked contiguously
        mt = (Mc.reshape(_NQ, 2, 2, _N, _N)       # (Q, b, h, i, j)
              .transpose(2, 4, 0, 1, 3)           # (h, j, Q, b, i)
              .reshape(128, _NQ * 128))
        mt = (mt.reshape(128, _NCHUNK, _CW).transpose(1, 0, 2))  # (chunk, p, f)
        mt = np.ascontiguousarray(mt).astype(ml_dtypes.bfloat16)

        maskd = np.zeros((128, _NP), np.float32)
        maskd[64 * b + 8 * c + s_loc, p] = 1.0   # v's node index is GLOBAL s

        Tp = T_full[8 * c + s_loc, t_loc]                     # [512]
        tmt = np.where(b == 0, Tp, 0.0).astype(np.float32)[None, :]
        tmb = np.where(b == 1, Tp, 0.0).astype(np.float32)[None, :]

        in_maps.append({"mt": mt, "maskd": maskd,
                        "tmt": np.ascontiguousarray(tmt),
                        "tmb": np.ascontiguousarray(tmb), "e2": e2})
    return in_maps


def kernel(x, weights_t, weights_r, r_zeros, r_const):
    from concourse.bass_utils import run_bass_kernel_spmd

    x = np.asarray(x, np.float32)
    weights_t = np.asarray(weights_t, np.float32)
    r_const = np.asarray(r_const, np.float32)
    r_zeros_np = np.asarray(r_zeros)
    if np.any(r_zeros_np):
        r_const = (np.asarray(weights_r, np.float32)
                   * r_zeros_np.astype(np.float32) + r_const)

    nc = _get_program()
    in_maps = _build_in_maps(x, weights_t, r_const)
    res = run_bass_kernel_spmd(nc, in_maps, core_ids=list(range(_NCORES)))
    parts = np.stack([r["out"][0, :] for r in res.results])  # [8, 64]
    return parts.sum(axis=0, dtype=np.float64).astype(np.float32)


# revision 16
# speedup vs baseline: 1.4977x; 1.0444x over previous
"""Trainium2 Bass kernel for nn_DegreePrediction (RBC via batched Perron vectors).

Math: M[s,t] = weights_r*r_zeros + r_const is positive column-stochastic
(columns sum to 1); its eigenvalue-1 right eigenvector is the Perron
vector and rbc[n] = sum_{s,t} T[s,t]/v[s,t,s] * v[s,t,n] is scale-free in
v.  v ~= M^2 @ ones to ~lambda2^2 ~ 0.4% << the 2e-2 gate, so two batched
mat-vec sweeps suffice (no squarings, no transposes).

Layout trick: each core's 512 matrices are uploaded TRANSPOSED in bf16,
two per 128-partition stack: MT[j+64h, 64q+i] = M_{2q+h}[i,j].  With
lhsT = a [128,128] MT block (stationary operand) both sweeps keep their
results in the PARTITION dim:
  pass A: rhs = ones-blocks [128,2]       -> out[m,n] = rowsums w_p[m]
  pass B: rhs = block-diag w cols [128,4] -> out[m,n] = v_p[m]
LDWEIGHTS/MATMUL pairs pipeline through the PE reorder window (~30ns per
block), so the kernel is DMA-paced: chunks are stored contiguously in
DRAM and streamed in order, and the pipeline is split in column halves
so pass B of half 0 and its tail overlap the DMA of half 1.  The
denominator row v_p[s_p] is gathered with a host mask + ones-matmul;
reciprocal runs on ACT (table preloaded during the DMA window; the DVE
iterative divide on a 1-partition row costs 3.3us).

Sharding: pairs split by s across 8 cores; host sums the partials.
"""

import numpy as np

_N = 64
_NCORES = 8
_NP = 512          # pairs per core
_NQ = 128          # double-stacks (4 pairs each)
_NCHUNK = 4        # DMA chunks of MT
_CW = _NQ * 128 // _NCHUNK   # MT cols per chunk (2048)
_QPC = _NQ // _NCHUNK        # double-stacks per chunk (16)

_cached = {}


def _build_program():
    import concourse.tile as tile
    from concourse import bacc, mybir
    from contextlib import ExitStack

    f32 = mybir.dt.float32
    bf16 = mybir.dt.bfloat16
    fp8 = mybir.dt.float8e4
    AF = mybir.ActivationFunctionType
    nc = bacc.Bacc("TRN2", target_bir_lowering=False, debug=False)
    mt_in = nc.dram_tensor("mt", [_NCHUNK, 128, _CW], bf16,
                           kind="ExternalInput").ap()
    maskd_in = nc.dram_tensor("maskd", [128, _NP], f32, kind="ExternalInput").ap()
    tmt_in = nc.dram_tensor("tmt", [1, _NP], f32, kind="ExternalInput").ap()
    tmb_in = nc.dram_tensor("tmb", [1, _NP], f32, kind="ExternalInput").ap()
    e2_in = nc.dram_tensor("e2", [128, _N], f32, kind="ExternalInput").ap()
    out_dram = nc.dram_tensor("out", [1, _N], f32, kind="ExternalOutput").ap()

    with tile.TileContext(nc) as tc:
        with ExitStack() as ctx:
            consts = ctx.enter_context(tc.tile_pool(name="consts", bufs=1))
            psum = ctx.enter_context(tc.tile_pool(name="psum", bufs=1, space="PSUM"))

            # ---- stream MT chunks first (contiguous, in order) ----
            mtc = []
            for d in range(_NCHUNK):
                t = consts.tile([128, _CW], bf16, tag=f"mt{d}")
                nc.sync.dma_start(out=t[:, :], in_=mt_in[d, :, :])
                mtc.append(t)

            # ---- small inputs (tail-only) ----
            maskd = consts.tile([128, _NP], f32)
            nc.sync.dma_start(out=maskd[:, :], in_=maskd_in[:, :])
            tmt = consts.tile([1, _NP], f32)
            nc.sync.dma_start(out=tmt[:, :], in_=tmt_in[:, :])
            tmb = consts.tile([1, _NP], f32)
            nc.sync.dma_start(out=tmb[:, :], in_=tmb_in[:, :])
            e2 = consts.tile([128, _N], f32)
            nc.sync.dma_start(out=e2[:, :], in_=e2_in[:, :])

            ones2 = consts.tile([128, 2], bf16)
            nc.vector.memset(ones2[:, :], 0.0)
            nc.vector.memset(ones2[0:64, 0:1], 1.0)
            nc.vector.memset(ones2[64:128, 1:2], 1.0)
            ones128 = consts.tile([128, 1], bf16)
            nc.vector.memset(ones128[:, :], 1.0)
            one1 = consts.tile([1, 1], f32)
            nc.vector.memset(one1[:, :], 1.0)
            etop = consts.tile([1, 128], bf16)
            nc.vector.memset(etop[:, :], 0.0)
            nc.vector.memset(etop[0:1, 0:64], 1.0)
            ebot = consts.tile([1, 128], bf16)
            nc.vector.memset(ebot[:, :], 0.0)
            nc.vector.memset(ebot[0:1, 64:128], 1.0)
            L = consts.tile([128, 4 * _NQ], bf16)
            nc.vector.memset(L[:, :], 0.0)
            # hoist the ACT ln/exp table load into the DMA window
            scratch = consts.tile([1, 1], f32)
            with tc.high_priority():
                nc.scalar.activation(out=scratch[:, :], in_=one1[:, :],
                                     func=AF.Ln)

            WW = psum.tile([128, 2 * _NQ], f32, tag="WW")
            VV = psum.tile([128, _NP], f32, tag="VV")
            DPS = psum.tile([1, _NP], f32, tag="DPS")
            CB = psum.tile([128, _NP], f32, tag="CB")
            WWv = WW[:, :].rearrange("p (q two) -> p q two", two=2)
            Lv = L[:, :].rearrange("p (q four) -> p q four", four=4)
            dmm = consts.tile([128, _NP], bf16)
            dinv = consts.tile([1, _NP], f32)
            ct = consts.tile([1, _NP], bf16)
            cb = consts.tile([1, _NP], bf16)
            cbs = consts.tile([128, _NP], f32)
            vc = consts.tile([128, _NP], f32)
            r1h = []
            for h in (0, 1):
                r1t = consts.tile([128, 1], f32, tag=f"r1{h}")
                r1h.append(r1t)

            def sweepA(h):
                for Q in range(64 * h, 64 * h + 64):
                    d, r = Q // _QPC, Q % _QPC
                    nc.tensor.matmul(
                        out=WW[:, 2 * Q:2 * Q + 2],
                        lhsT=mtc[d][:, 128 * r:128 * r + 128],
                        rhs=ones2[:, :], start=True, stop=True)

            def lbuild(h):
                qs = slice(64 * h, 64 * h + 64)
                nc.vector.tensor_copy(out=Lv[0:64, qs, 0], in_=WWv[0:64, qs, 0])
                nc.vector.tensor_copy(out=Lv[64:128, qs, 1], in_=WWv[0:64, qs, 1])
                nc.vector.tensor_copy(out=Lv[0:64, qs, 2], in_=WWv[64:128, qs, 0])
                nc.vector.tensor_copy(out=Lv[64:128, qs, 3], in_=WWv[64:128, qs, 1])

            def sweepB(h):
                for Q in range(64 * h, 64 * h + 64):
                    d, r = Q // _QPC, Q % _QPC
                    nc.tensor.matmul(
                        out=VV[:, 4 * Q:4 * Q + 4],
                        lhsT=mtc[d][:, 128 * r:128 * r + 128],
                        rhs=L[:, 4 * Q:4 * Q + 4], start=True, stop=True)

            def tail_dve(h):
                sl = slice(256 * h, 256 * h + 256)
                nc.vector.tensor_mul(out=dmm[:, sl], in0=VV[:, sl],
                                     in1=maskd[:, sl])

            def tail_pe_d(h):
                sl = slice(256 * h, 256 * h + 256)
                nc.tensor.matmul(out=DPS[:, sl], lhsT=ones128[:, :],
                                 rhs=dmm[:, sl], start=True, stop=True)

            def tail_coef(h):
                sl = slice(256 * h, 256 * h + 256)
                # 1/d = exp(-ln d) on ACT: d > 0 (Perron), and the DVE
                # iterative divide on a 1-partition row costs 8 cyc/elem.
                nc.scalar.activation(out=dinv[:, sl], in_=DPS[:, sl],
                                     func=AF.Ln)
                nc.scalar.activation(out=dinv[:, sl], in_=dinv[:, sl],
                                     func=AF.Exp, scale=-1.0)
                nc.vector.tensor_mul(out=ct[:, sl], in0=tmt[:, sl],
                                     in1=dinv[:, sl])
                nc.vector.tensor_mul(out=cb[:, sl], in0=tmb[:, sl],
                                     in1=dinv[:, sl])

            def tail_pe_cb(h):
                sl = slice(256 * h, 256 * h + 256)
                nc.tensor.matmul(out=CB[:, sl], lhsT=etop[:, :], rhs=ct[:, sl],
                                 start=True, stop=False)
                nc.tensor.matmul(out=CB[:, sl], lhsT=ebot[:, :], rhs=cb[:, sl],
                                 start=False, stop=True)

            def tail_fin(h):
                sl = slice(256 * h, 256 * h + 256)
                nc.scalar.copy(out=cbs[:, sl], in_=CB[:, sl])
                nc.vector.tensor_mul(out=vc[:, sl], in0=VV[:, sl],
                                     in1=cbs[:, sl])
                nc.vector.tensor_reduce(
                    out=r1h[h][:, :], in_=vc[:, sl],
                    axis=mybir.AxisListType.X, op=mybir.AluOpType.add)

            sweepA(0)
            lbuild(0)
            sweepB(0)
            tail_dve(0)
            sweepA(1)          # PE: runs while half-0 tail DVE work proceeds
            tail_pe_d(0)
            lbuild(1)
            tail_coef(0)
            tail_pe_cb(0)
            sweepB(1)
            tail_fin(0)
            tail_dve(1)
            tail_pe_d(1)
            tail_coef(1)
            tail_pe_cb(1)
            tail_fin(1)

            r1 = consts.tile([128, 1], f32)
            nc.vector.tensor_add(out=r1[:, :], in0=r1h[0][:, :], in1=r1h[1][:, :])
            # fold halves AND transpose to a row in one matmul:
            # FR[0,n] = sum_k r1[k]*E2[k,n] = r1[n] + r1[64+n]
            FR = psum.tile([1, _N], f32, tag="FR")
            nc.tensor.matmul(out=FR[:, :], lhsT=r1[:, :], rhs=e2[:, :],
                             start=True, stop=True)
            out_sb = consts.tile([1, _N], f32)
            nc.scalar.copy(out=out_sb[:, :], in_=FR[:, :])
            nc.sync.dma_start(out=out_dram[:, :], in_=out_sb[:, :])
    nc.compile()
    return nc


def _get_program():
    if "nc" not in _cached:
        _cached["nc"] = _build_program()
    return _cached["nc"]


def _build_in_maps(x, weights_t, r_const):
    """Host-side layouts for all 8 cores."""
    import ml_dtypes

    M_all = r_const.reshape(_N * _N, _N, _N)
    i = np.arange(_N)
    r_diag = r_const[i[:, None], i[None, :], i[:, None], i[:, None]]
    T_full = (x * weights_t * r_diag).astype(np.float32)      # [64, 64]

    e2 = np.zeros((128, _N), np.float32)
    e2[np.arange(128), np.arange(128) % _N] = 1.0

    p = np.arange(_NP)
    b = (p >> 1) & 1                                          # stack-half of pair
    s_loc = p >> 6
    t_loc = p & 63

    in_maps = []
    for c in range(_NCORES):
        Mc = np.asarray(M_all[_NP * c:_NP * (c + 1)], np.float32)  # (p,i,j)
        # MT[j+64h, 64(2Q+b)+i] = Mc[4Q+2b+h, i, j], then chunked contiguously
        mt = (Mc.reshape(_NQ, 2, 2, _N, _N)       # (Q, b, h, i, j)
              .transpose(2, 4, 0, 1, 3)           # (h, j, Q, b, i)
              .reshape(128, _NQ * 128))
        mt = (mt.reshape(128, _NCHUNK, _CW).transpose(1, 0, 2))  # (chunk, p, f)
        mt = np.ascontiguousarray(mt).astype(ml_dtypes.bfloat16)

        maskd = np.zeros((128, _NP), np.float32)
        maskd[64 * b + 8 * c + s_loc, p] = 1.0   # v's node index is GLOBAL s

        Tp = T_full[8 * c + s_loc, t_loc]                     # [512]
        tmt = np.where(b == 0, Tp, 0.0).astype(np.float32)[None, :]
        tmb = np.where(b == 1, Tp, 0.0).astype(np.float32)[None, :]

        in_maps.append({"mt": mt, "maskd": maskd,
                        "tmt": np.ascontiguousarray(tmt),
                        "tmb": np.ascontiguousarray(tmb), "e2": e2})
    return in_maps


def kernel(x, weights_t, weights_r, r_zeros, r_const):
    from concourse.bass_utils import run_bass_kernel_spmd

    x = np.asarray(x, np.float32)
    weights_t = np.asarray(weights_t, np.float32)
    r_const = np.asarray(r_const, np.float32)
    r_zeros_np = np.asarray(r_zeros)
    if np.any(r_zeros_np):
        r_const = (np.asarray(weights_r, np.float32)
                   * r_zeros_np.astype(np.float32) + r_const)

    nc = _get_program()
    in_maps = _build_in_maps(x, weights_t, r_const)
    res = run_bass_kernel_spmd(nc, in_maps, core_ids=list(range(_NCORES)))
    parts = np.stack([r["out"][0, :] for r in res.results])  # [8, 64]
    return parts.sum(axis=0, dtype=np.float64).astype(np.float32)
